# revision 1
# baseline (speedup 1.0000x reference)
"""MoE linear-regression router kernel for Trainium2 (8 NeuronCores, data-parallel).

Math (only the part of the reference that affects the output):
    nl  = x @ Wn.T + bn            [B, 64]
    top8 per row of nl -> masked softmax -> routing weights
    eo  = x @ We.T                 [B, 64]
    y   = sum(eo * weights, -1)    [B, 1]
(Wg, bg, noise feed a deleted intermediate in the reference; they do not
affect the output.)

Per-core plan (fp32 everywhere; router selection must match fp32 reference):
  - x sliced along batch over 8 cores (8192 rows each), weights replicated.
  - groups of 512 tokens: DMA x -> PE-transpose to xT chunks [128d, 512b]
    -> one fused matmul vs stacked [Wn|We].T (stationary) accumulating over
    8 K-chunks into PSUM [128e, 512b]
  - ScalarE copy PSUM->SBUF with fused per-partition bias [bn;0]
  - PE re-transpose to token-major [128b, 128(nl|eo)] tiles
  - routing per tile: DVE max8 -> threshold mask -> ACT exp (+accumulated Z
    from the top-8 values) -> weighted sum -> final = num/Z
  - final column transposed once and stored with a single contiguous DMA.
"""
from contextlib import ExitStack

import numpy as np

import concourse.mybir as mybir
import concourse.tile as tile
from concourse import bacc
from concourse.bass_utils import run_bass_kernel_spmd

F32 = mybir.dt.float32
AF = mybir.ActivationFunctionType
ALU = mybir.AluOpType

B, D, E, NCORES = 65536, 1024, 64, 8
GROUP = 512
NK = D // 128  # K-chunks


def build_kernel(b_local, repeat=1, pipelined=True, copy_split=5, tp4=False,
                 diag_mm1=False, diag_notrans=False, interleave=False,
                 xtp_bufs=16, stage_bufs=2, tok_bufs=4, pool_mode="stack"):
    ng = b_local // GROUP
    nf = ng * 4  # token-major tiles (= output columns in the staging tile)
    assert nf <= 128

    nc = bacc.Bacc("TRN2", target_bir_lowering=False)
    x_d = nc.dram_tensor("x", [b_local, D], F32, kind="ExternalInput")
    wt_d = nc.dram_tensor("wt", [D, 128], F32, kind="ExternalInput")
    bnst_d = nc.dram_tensor("bnst", [128, 1], F32, kind="ExternalInput")
    ident_d = nc.dram_tensor("ident", [128, 128], F32, kind="ExternalInput")
    y_d = nc.dram_tensor("y", [b_local, 1], F32, kind="ExternalOutput")

    with tile.TileContext(nc, pool_alloc_mode=pool_mode) as tc:
        with ExitStack() as ctx:
            consts = ctx.enter_context(tc.tile_pool(name="consts", bufs=1))
            xin = ctx.enter_context(tc.tile_pool(name="xin", bufs=3))
            xtp = ctx.enter_context(tc.tile_pool(name="xtp", bufs=xtp_bufs))
            rt = ctx.enter_context(tc.tile_pool(name="rt", bufs=3))
            ps_stage = ctx.enter_context(
                tc.tile_pool(name="ps_stage", bufs=stage_bufs, space="PSUM"))
            ps_mm = ctx.enter_context(
                tc.tile_pool(name="ps_mm", bufs=2, space="PSUM"))
            ps_tok = ctx.enter_context(
                tc.tile_pool(name="ps_tok", bufs=tok_bufs, space="PSUM"))

            # identity first (needed by the very first transpose); weight and
            # bias DMAs are deferred until after the first x-tile DMA so the
            # 2 MB x load isn't queued behind them at kernel start
            ident_t = consts.tile([128, 128], F32)
            nc.sync.dma_start(out=ident_t, in_=ident_d[:, :])
            wt_t = consts.tile([128, NK, 128], F32)
            bnst_t = consts.tile([128, 1], F32)

            def emit_const_dmas():
                nc.sync.dma_start(out=wt_t, in_=wt_d[:, :].rearrange(
                    "(k p) e -> p k e", p=128))
                nc.sync.dma_start(out=bnst_t, in_=bnst_d[:, :])

            z_all = consts.tile([128, nf], F32)
            num_all = consts.tile([128, nf], F32)

            def emit_transpose(dst, src):
                # dst = src.T for [128,128] tiles; tp4 packs it as four
                # concurrent [64,64] quadrant transposes (disjoint row/col
                # groups of the PE array)
                if not tp4:
                    nc.tensor.transpose(dst, src, ident_t[:])
                    return
                for I in (0, 64):
                    for J in (0, 64):
                        nc.tensor.transpose(
                            dst[I:I + 64, J:J + 64],
                            src[J:J + 64, I:I + 64],
                            ident_t[J:J + 64, J:J + 64],
                            tile_position=(J, I))

            def emit_load_transpose(g):
                xg = xin.tile([128, 4, D], F32, tag="xg")
                nc.sync.dma_start(
                    out=xg,
                    in_=x_d[g * GROUP:(g + 1) * GROUP, :].rearrange(
                        "(j p) d -> p j d", p=128))
                # transpose x into xT chunks [128d, 512b]
                xts = []
                for k in range(NK):
                    xt = xtp.tile([128, 512], F32, tag="xt")
                    if diag_notrans:
                        nc.gpsimd.memset(xt[:], 0.0)
                        xts.append(xt)
                        continue
                    st = ps_stage.tile([128, 512], F32, tag="st")
                    for j in range(4):
                        emit_transpose(st[:, j * 128:(j + 1) * 128],
                                       xg[:, j, k * 128:(k + 1) * 128])
                    if k < copy_split:
                        nc.scalar.activation(xt[:], st[:], AF.Copy)
                    else:
                        nc.vector.tensor_copy(xt[:], st[:])
                    xts.append(xt)
                return xts

            def emit_compute(g, xts):
                # fused router+expert matmul: [Wn|We].T stationary
                pm = ps_mm.tile([128, 512], F32, tag="pm")
                nmm = 1 if diag_mm1 else NK
                for k in range(nmm):
                    nc.tensor.matmul(pm[:], wt_t[:, k, :], xts[k][:],
                                     start=(k == 0), stop=(k == nmm - 1))

                # bias fused into the PSUM->SBUF copy (rows 0:64 get bn)
                nbeo = rt.tile([128, 512], F32, tag="nbeo")
                nc.scalar.activation(nbeo[:], pm[:], AF.Identity,
                                     bias=bnst_t[:, 0:1])

                pg = rt.tile([128, 4, E], F32, tag="pg")
                for j in range(4):
                    col = 4 * g + j
                    pt = ps_tok.tile([128, 128], F32, tag="pt")
                    emit_transpose(pt[:], nbeo[:, j * 128:(j + 1) * 128])
                    nl = pt[:, 0:E]
                    eo = pt[:, E:2 * E]

                    m8 = rt.tile([128, 8], F32, tag="m8")
                    nc.vector.max(m8[:], nl)
                    w = rt.tile([128, E], F32, tag="w")
                    nc.scalar.activation(w[:], nl, AF.Exp)
                    e8 = rt.tile([128, 8], F32, tag="e8")
                    nc.scalar.activation(e8[:], m8[:], AF.Exp,
                                         accum_out=z_all[:, col:col + 1])
                    mask = rt.tile([128, E], F32, tag="mask")
                    nc.vector.tensor_scalar(
                        out=mask[:], in0=nl, scalar1=m8[:, 7:8], scalar2=None,
                        op0=ALU.is_ge)
                    wm = rt.tile([128, E], F32, tag="wm")
                    nc.vector.tensor_tensor(out=wm[:], in0=w[:], in1=mask[:],
                                            op=ALU.mult)
                    nc.vector.tensor_tensor(out=pg[:, j, :], in0=wm[:], in1=eo,
                                            op=ALU.mult)
                nc.vector.tensor_reduce(
                    out=num_all[:, 4 * g:4 * g + 4], in_=pg[:],
                    axis=mybir.AxisListType.X, op=ALU.add)

            def emit_chunk(g, xg, k):
                # one d-chunk: 4 quadrant transposes into a stage bank + copy
                xt = xtp.tile([128, 512], F32, tag="xt")
                st = ps_stage.tile([128, 512], F32, tag="st")
                for j in range(4):
                    emit_transpose(st[:, j * 128:(j + 1) * 128],
                                   xg[:, j, k * 128:(k + 1) * 128])
                if k < copy_split:
                    nc.scalar.activation(xt[:], st[:], AF.Copy)
                else:
                    nc.vector.tensor_copy(xt[:], st[:])
                return xt

            def emit_dma(g):
                xg = xin.tile([128, 4, D], F32, tag="xg")
                nc.sync.dma_start(
                    out=xg,
                    in_=x_d[g * GROUP:(g + 1) * GROUP, :].rearrange(
                        "(j p) d -> p j d", p=128))
                return xg

            def emit_tail(g, pm):
                nbeo = rt.tile([128, 512], F32, tag="nbeo")
                nc.scalar.activation(nbeo[:], pm[:], AF.Identity,
                                     bias=bnst_t[:, 0:1])
                pg = rt.tile([128, 4, E], F32, tag="pg")
                for j in range(4):
                    col = 4 * g + j
                    pt = ps_tok.tile([128, 128], F32, tag="pt")
                    emit_transpose(pt[:], nbeo[:, j * 128:(j + 1) * 128])
                    nl = pt[:, 0:E]
                    eo = pt[:, E:2 * E]
                    m8 = rt.tile([128, 8], F32, tag="m8")
                    nc.vector.max(m8[:], nl)
                    w = rt.tile([128, E], F32, tag="w")
                    nc.scalar.activation(w[:], nl, AF.Exp)
                    e8 = rt.tile([128, 8], F32, tag="e8")
                    nc.scalar.activation(e8[:], m8[:], AF.Exp,
                                         accum_out=z_all[:, col:col + 1])
                    mask = rt.tile([128, E], F32, tag="mask")
                    nc.vector.tensor_scalar(
                        out=mask[:], in0=nl, scalar1=m8[:, 7:8], scalar2=None,
                        op0=ALU.is_ge)
                    wm = rt.tile([128, E], F32, tag="wm")
                    nc.vector.tensor_tensor(out=wm[:], in0=w[:], in1=mask[:],
                                            op=ALU.mult)
                    nc.vector.tensor_tensor(out=pg[:, j, :], in0=wm[:], in1=eo,
                                            op=ALU.mult)
                nc.vector.tensor_reduce(
                    out=num_all[:, 4 * g:4 * g + 4], in_=pg[:],
                    axis=mybir.AxisListType.X, op=ALU.add)

            glist = [g for _ in range(repeat) for g in range(ng)]
            if interleave:
                # chunk-level interleave: PE stream alternates T(g,k) with
                # MM(g-1,k) so stage-copy latency never stalls the PE
                prev = None  # (g, xts)
                for i, g in enumerate(glist):
                    xg = emit_dma(g)
                    if i == 0:
                        emit_const_dmas()
                    xts = []
                    if prev is not None:
                        pm = ps_mm.tile([128, 512], F32, tag="pm")
                    else:
                        pm = None
                    for k in range(NK):
                        xts.append(emit_chunk(g, xg, k))
                        if prev is not None:
                            pg_, pxts = prev
                            nc.tensor.matmul(pm[:], wt_t[:, k, :],
                                             pxts[k][:], start=(k == 0),
                                             stop=(k == NK - 1),
                                             skip_group_check=True)
                    if prev is not None:
                        emit_tail(prev[0], pm)
                    prev = (g, xts)
                g, xts = prev
                pm = ps_mm.tile([128, 512], F32, tag="pm")
                for k in range(NK):
                    nc.tensor.matmul(pm[:], wt_t[:, k, :], xts[k][:],
                                     start=(k == 0), stop=(k == NK - 1),
                                     skip_group_check=True)
                emit_tail(g, pm)
            elif pipelined:
                # one-group software skew: PE transposes group g+1 while the
                # matmuls of group g wait on g's PSUM->SBUF copies
                pending = None
                for i, g in enumerate(glist):
                    xts = emit_load_transpose(g)
                    if i == 0:
                        emit_const_dmas()
                    if pending is not None:
                        emit_compute(*pending)
                    pending = (g, xts)
                emit_compute(*pending)
            else:
                for i, g in enumerate(glist):
                    xts = emit_load_transpose(g)
                    if i == 0:
                        emit_const_dmas()
                    emit_compute(g, xts)

            zinv = consts.tile([128, nf], F32)
            nc.vector.reciprocal(zinv[:], z_all[:])
            ostage = consts.tile([128, nf], F32)
            nc.vector.tensor_tensor(out=ostage[:], in0=num_all[:], in1=zinv[:],
                                    op=ALU.mult)
            po = ps_mm.tile([nf, 128], F32, tag="pm")
            nc.tensor.transpose(po[:], ostage[:], ident_t[:])
            ofin = consts.tile([nf, 128], F32)
            nc.vector.tensor_copy(ofin[:], po[:])
            nc.sync.dma_start(
                out=y_d[:, :].rearrange("(f p) one -> f (p one)", p=128),
                in_=ofin[:])
    nc.finalize()
    return nc


def _prep_weights(Wn, bn, We):
    wt = np.ascontiguousarray(
        np.concatenate([Wn, We], axis=0).T.astype(np.float32))  # [D, 128]
    bnst = np.zeros((128, 1), np.float32)
    bnst[:E, 0] = bn.astype(np.float32)
    ident = np.eye(128, dtype=np.float32)
    return wt, bnst, ident


_BUILD_CACHE = {}


def run(x, Wn, bn, We, b_local=None, cores=None, trace=False, nruns=1,
        verbose=False):
    import time as _time
    x = np.ascontiguousarray(np.asarray(x, np.float32))
    n = x.shape[0]
    if cores is None:
        cores = list(range(NCORES))
    if b_local is None:
        b_local = n // len(cores)
    assert n == b_local * len(cores) and b_local % GROUP == 0, (n, b_local)
    wt, bnst, ident = _prep_weights(np.asarray(Wn), np.asarray(bn),
                                    np.asarray(We))
    t0 = _time.time()
    if b_local not in _BUILD_CACHE:
        _BUILD_CACHE[b_local] = build_kernel(b_local, interleave=True)
    nc = _BUILD_CACHE[b_local]
    t_build = _time.time() - t0
    in_maps = []
    for i in range(len(cores)):
        in_maps.append({
            "x": x[i * b_local:(i + 1) * b_local],
            "wt": wt, "bnst": bnst, "ident": ident,
        })
    walls = []
    for r in range(nruns):
        t0 = _time.time()
        res = run_bass_kernel_spmd(nc, in_maps, core_ids=cores, trace=trace)
        walls.append(_time.time() - t0)
    if verbose:
        print(f"  build={t_build:.1f}s walls={[f'{w:.2f}' for w in walls]}")
    y = np.concatenate([r["y"] for r in res.results], axis=0)
    return y, res


def kernel(x, Wg, bg, Wn, bn, We, noise):
    y, _ = run(x, Wn, bn, We)
    return y



# revision 2
# speedup vs baseline: 1.8297x; 1.8297x over previous
"""MoE linear-regression router kernel for Trainium2 (8 NeuronCores, data-parallel).

Math (only the part of the reference that affects the output):
    nl  = x @ Wn.T + bn            [B, 64]
    top8 per row of nl -> masked softmax -> routing weights
    eo  = x @ We.T                 [B, 64]
    y   = sum(eo * weights, -1)    [B, 1]
(Wg, bg, noise feed a deleted intermediate in the reference; they do not
affect the output.)

Implementation (per core, x sliced along batch over 8 cores):
  - groups of 512 tokens; per 128-d chunk: PE-transpose x (fp32) into PSUM,
    ACT/DVE copy to SBUF fp32.
  - bf16 3-pass matmul against stacked [Wn|We] weights split into bf16
    hi/lo on the host:  hi@Wh + hi@Wl + xl@Wh, accumulated in PSUM.
      hi = stride-2 bf16 view of the fp32 tile (upper 2 bytes = truncated
           bf16) -- no cast instruction needed.
      xl = xt - hi, computed on GpSimd (SBUF-only), output bf16.
    Error vs exact fp32 is ~2^-17 relative on the logits; measured
    end-to-end rel err ~3.6e-3 (tolerance 2e-2).
  - bias fused into the PSUM->SBUF copy; PE re-transpose to token-major
    tiles; DVE top-8 (max8) -> threshold mask -> ACT exp (+accumulated Z)
    -> weighted sum -> final = num/Z; one contiguous output DMA.
"""
from contextlib import ExitStack

import numpy as np
import ml_dtypes

import concourse.mybir as mybir
import concourse.tile as tile
from concourse import bacc
from concourse.bass_utils import run_bass_kernel_spmd

F32 = mybir.dt.float32
BF16 = mybir.dt.bfloat16
AF = mybir.ActivationFunctionType
ALU = mybir.AluOpType

B, D, E, NCORES = 65536, 1024, 64, 8
GROUP = 512
NK = D // 128  # contraction chunks


def _hi_view(xt):
    """Upper-2-byte bf16 view of an fp32 [128, 512] tile (truncated bf16)."""
    return xt[:].bitcast(BF16).rearrange("p (f two) -> p f two", two=2)[:, :, 1]


def build_kernel(b_local, repeat=1, copy_dve=2, xtp_bufs=16, stage_bufs=2,
                 tok_bufs=4):
    ng = b_local // GROUP
    nf = ng * 4  # output columns in the staging tile
    assert nf <= 128

    nc = bacc.Bacc("TRN2", target_bir_lowering=False)
    x_d = nc.dram_tensor("x", [b_local, D], F32, kind="ExternalInput")
    wth_d = nc.dram_tensor("wth", [D, 128], BF16, kind="ExternalInput")
    wtl_d = nc.dram_tensor("wtl", [D, 128], BF16, kind="ExternalInput")
    bnst_d = nc.dram_tensor("bnst", [128, 1], F32, kind="ExternalInput")
    ident_d = nc.dram_tensor("ident", [128, 128], F32, kind="ExternalInput")
    y_d = nc.dram_tensor("y", [b_local, 1], F32, kind="ExternalOutput")

    with tile.TileContext(nc, pool_alloc_mode="stack") as tc:
        with ExitStack() as ctx:
            consts = ctx.enter_context(tc.tile_pool(name="consts", bufs=1))
            xin = ctx.enter_context(tc.tile_pool(name="xin", bufs=3))
            xtp = ctx.enter_context(tc.tile_pool(name="xtp", bufs=xtp_bufs))
            rt = ctx.enter_context(tc.tile_pool(name="rt", bufs=3))
            ps_stage = ctx.enter_context(
                tc.tile_pool(name="ps_stage", bufs=stage_bufs, space="PSUM"))
            ps_mm = ctx.enter_context(
                tc.tile_pool(name="ps_mm", bufs=2, space="PSUM"))
            ps_tok = ctx.enter_context(
                tc.tile_pool(name="ps_tok", bufs=tok_bufs, space="PSUM"))

            # identity first (needed by the very first transpose); weight and
            # bias DMAs are deferred until after the first x-tile DMA so the
            # 2 MB x load isn't queued behind them at kernel start
            ident_t = consts.tile([128, 128], F32)
            nc.sync.dma_start(out=ident_t, in_=ident_d[:, :])
            wth_t = consts.tile([128, NK, 128], BF16)
            wtl_t = consts.tile([128, NK, 128], BF16)
            bnst_t = consts.tile([128, 1], F32)

            def emit_const_dmas():
                nc.sync.dma_start(out=wth_t, in_=wth_d[:, :].rearrange(
                    "(k p) e -> p k e", p=128))
                nc.sync.dma_start(out=wtl_t, in_=wtl_d[:, :].rearrange(
                    "(k p) e -> p k e", p=128))
                nc.sync.dma_start(out=bnst_t, in_=bnst_d[:, :])

            z_all = consts.tile([128, nf], F32)
            num_all = consts.tile([128, nf], F32)

            def emit_dma(g):
                xg = xin.tile([128, 4, D], F32, tag="xg")
                nc.sync.dma_start(
                    out=xg,
                    in_=x_d[g * GROUP:(g + 1) * GROUP, :].rearrange(
                        "(j p) d -> p j d", p=128))
                return xg

            def emit_chunk(g, xg, k):
                st = ps_stage.tile([128, 512], F32, tag="st")
                for j in range(4):
                    nc.tensor.transpose(st[:, j * 128:(j + 1) * 128],
                                        xg[:, j, k * 128:(k + 1) * 128],
                                        ident_t[:])
                xt = xtp.tile([128, 512], F32, tag="xt")
                if k < NK - copy_dve:
                    nc.scalar.activation(xt[:], st[:], AF.Copy)
                else:
                    nc.vector.tensor_copy(xt[:], st[:])
                h = _hi_view(xt)
                xl = xtp.tile([128, 512], BF16, tag="xl")
                nc.gpsimd.tensor_tensor(out=xl[:], in0=xt[:], in1=h,
                                        op=ALU.subtract)
                return (h, xl)

            def emit_mms(pm, k, ops, first, last):
                h, xl = ops
                nc.tensor.matmul(pm[:], wth_t[:, k, :], h, start=first,
                                 stop=False, skip_group_check=True)
                nc.tensor.matmul(pm[:], wtl_t[:, k, :], h, start=False,
                                 stop=False, skip_group_check=True)
                nc.tensor.matmul(pm[:], wth_t[:, k, :], xl[:], start=False,
                                 stop=last, skip_group_check=True)

            def emit_tail(g, pm):
                nbeo = rt.tile([128, 512], F32, tag="nbeo")
                nc.scalar.activation(nbeo[:], pm[:], AF.Identity,
                                     bias=bnst_t[:, 0:1])
                pg = rt.tile([128, 4, E], F32, tag="pg")
                for j in range(4):
                    col = 4 * g + j
                    pt = ps_tok.tile([128, 128], F32, tag="pt")
                    nc.tensor.transpose(pt[:],
                                        nbeo[:, j * 128:(j + 1) * 128],
                                        ident_t[:])
                    nl = pt[:, 0:E]
                    eo = pt[:, E:2 * E]
                    m8 = rt.tile([128, 8], F32, tag="m8")
                    nc.vector.max(m8[:], nl)
                    w = rt.tile([128, E], F32, tag="w")
                    nc.scalar.activation(w[:], nl, AF.Exp)
                    e8 = rt.tile([128, 8], F32, tag="e8")
                    nc.scalar.activation(e8[:], m8[:], AF.Exp,
                                         accum_out=z_all[:, col:col + 1])
                    mask = rt.tile([128, E], F32, tag="mask")
                    nc.vector.tensor_scalar(
                        out=mask[:], in0=nl, scalar1=m8[:, 7:8], scalar2=None,
                        op0=ALU.is_ge)
                    wm = rt.tile([128, E], F32, tag="wm")
                    nc.vector.tensor_tensor(out=wm[:], in0=w[:], in1=mask[:],
                                            op=ALU.mult)
                    nc.vector.tensor_tensor(out=pg[:, j, :], in0=wm[:],
                                            in1=eo, op=ALU.mult)
                nc.vector.tensor_reduce(
                    out=num_all[:, 4 * g:4 * g + 4], in_=pg[:],
                    axis=mybir.AxisListType.X, op=ALU.add)

            glist = [g for _ in range(repeat) for g in range(ng)]
            prev = None
            for i, g in enumerate(glist):
                xg = emit_dma(g)
                if i == 0:
                    emit_const_dmas()
                cur = []
                if prev is not None:
                    pm = ps_mm.tile([128, 512], F32, tag="pm")
                else:
                    pm = None
                for k in range(NK):
                    cur.append(emit_chunk(g, xg, k))
                    if prev is not None:
                        emit_mms(pm, k, prev[1][k], first=(k == 0),
                                 last=(k == NK - 1))
                if prev is not None:
                    emit_tail(prev[0], pm)
                prev = (g, cur)
            g, cur = prev
            pm = ps_mm.tile([128, 512], F32, tag="pm")
            for k in range(NK):
                emit_mms(pm, k, cur[k], first=(k == 0), last=(k == NK - 1))
            emit_tail(g, pm)

            zinv = consts.tile([128, nf], F32)
            nc.vector.reciprocal(zinv[:], z_all[:])
            ostage = consts.tile([128, nf], F32)
            nc.vector.tensor_tensor(out=ostage[:], in0=num_all[:],
                                    in1=zinv[:], op=ALU.mult)
            po = ps_mm.tile([nf, 128], F32, tag="pm")
            nc.tensor.transpose(po[:], ostage[:], ident_t[:])
            ofin = consts.tile([nf, 128], F32)
            nc.vector.tensor_copy(ofin[:], po[:])
            nc.sync.dma_start(
                out=y_d[:, :].rearrange("(f p) one -> f (p one)", p=128),
                in_=ofin[:])
    nc.finalize()
    return nc


def _prep_weights(Wn, bn, We):
    wt = np.ascontiguousarray(
        np.concatenate([Wn, We], axis=0).T.astype(np.float32))  # [D, 128]
    wth = wt.astype(ml_dtypes.bfloat16)
    wtl = (wt - wth.astype(np.float32)).astype(ml_dtypes.bfloat16)
    bnst = np.zeros((128, 1), np.float32)
    bnst[:E, 0] = bn.astype(np.float32)
    ident = np.eye(128, dtype=np.float32)
    return {"wth": np.ascontiguousarray(wth),
            "wtl": np.ascontiguousarray(wtl),
            "bnst": bnst, "ident": ident}


_BUILD_CACHE = {}


def run(x, Wn, bn, We, b_local=None, cores=None, trace=False, nruns=1,
        verbose=False):
    import time as _time
    x = np.ascontiguousarray(np.asarray(x, np.float32))
    n = x.shape[0]
    if cores is None:
        cores = list(range(NCORES))
    if b_local is None:
        b_local = n // len(cores)
    assert n == b_local * len(cores) and b_local % GROUP == 0, (n, b_local)
    wmap = _prep_weights(np.asarray(Wn), np.asarray(bn), np.asarray(We))
    t0 = _time.time()
    if b_local not in _BUILD_CACHE:
        _BUILD_CACHE[b_local] = build_kernel(b_local)
    nc = _BUILD_CACHE[b_local]
    t_build = _time.time() - t0
    in_maps = []
    for i in range(len(cores)):
        in_maps.append({"x": x[i * b_local:(i + 1) * b_local], **wmap})
    walls = []
    for r in range(nruns):
        t0 = _time.time()
        res = run_bass_kernel_spmd(nc, in_maps, core_ids=cores, trace=trace)
        walls.append(_time.time() - t0)
    if verbose:
        print(f"  build={t_build:.1f}s walls={[f'{w:.2f}' for w in walls]}")
    y = np.concatenate([r["y"] for r in res.results], axis=0)
    return y, res


def kernel(x, Wg, bg, Wn, bn, We, noise):
    y, _ = run(x, Wn, bn, We)
    return y
